# revision 1
# baseline (speedup 1.0000x reference)
"""AdaAPPNP (gated APPNP GNN) distributed Bass kernel for 8 TRN2 NeuronCores.

Strategy: node-sharded graph parallelism. Each core owns 12500 dst nodes.
Per hop the gated-APPNP update needs two spmms (A @ lg', A @ hc') over the
symmetric-normalized adjacency. The normalization is factored into node
scales s = deg^-1/2 so the edge loop is an unweighted segment-sum:
  spmm_hat(x) = s * (A @ (s * x)).
Both propagated matrices are packed into one bf16 table row [lg' | hc']
(128 feats, 256B) so one gather per edge serves both spmms. Per step the
8 shards are AllGathered into a replicated table; each core dma_gathers
its in-edges' source rows (int16 idx => gather per 25088-row block) and
reduces them per 128-dst window with TensorE: S^T @ E where S is a
one-hot edge->dst matrix built on-the-fly by VectorE (is_equal vs iota).
Pad edges get dst -1 so S zero-kills them.
"""

import numpy as np
import ml_dtypes

N = 100000
F = 512
H = 128
C = 64
K = 10
NCORE = 8
SH = 12500            # real nodes per core
NW = 98               # 128-dst windows per core
SHP = NW * 128        # padded shard rows (12544)
BLK = 2 * SHP         # gather block rows (25088 < 32768, int16-safe)
NB = 4                # gather blocks (core pairs)
WB = 3                # windows per gather batch
NBATCH = (NW + WB - 1) // WB  # 33
PADROW = SH           # block-local index of a guaranteed-zero row

bf16 = ml_dtypes.bfloat16

_last_exec_ns = None
TRACE = False


def _to_bf16_u16(a):
    return np.ascontiguousarray(a.astype(bf16)).view(np.uint16)


def _host_prep(features, W1, b1, W2, b2, init_weight_y, edge_index):
    src = edge_index[0].astype(np.int64)
    dst = edge_index[1].astype(np.int64)
    deg = np.bincount(dst, minlength=N).astype(np.float32)
    s = (1.0 / np.sqrt(np.clip(deg, 1.0, None))).astype(np.float32)

    # table row of a source node (core-major shard layout, SHP rows/core)
    trow = (src // SH) * SHP + (src % SH)
    blk = trow // BLK
    lidx = trow % BLK

    core = dst // SH
    d_local = dst - core * SH
    w = d_local // 128
    lane = d_local % 128

    # per (core, block, window) cell counts
    cell = (core * NB + blk) * NW + w
    counts = np.bincount(cell, minlength=NCORE * NB * NW).reshape(NCORE, NB, NW)
    P = (np.ceil(counts.max(axis=0) / 128).astype(np.int64) * 128)  # [NB, NW]
    P = np.maximum(P, 128)

    # stream order: batch-major, then block, then window
    # stream_off[B, w] = start of cell (B, w) in the per-core stream
    stream_off = np.zeros((NB, NW), np.int64)
    off = 0
    gather_info = []  # per (batch, B): (idx_off, idx_len)
    for b in range(NBATCH):
        ws = list(range(b * WB, min((b + 1) * WB, NW)))
        for B in range(NB):
            g_off = off
            for w_ in ws:
                stream_off[B, w_] = off
                off += int(P[B, w_])
            gather_info.append((b, B, g_off, off - g_off))
    TOT = off
    TILES = TOT // 128

    # fill per-core streams (vectorized by sorting)
    idx_streams = np.full((NCORE, TOT), PADROW, np.int16)
    dst_streams = np.full((NCORE, TOT), -1.0, np.float32)
    order = np.lexsort((lane, w, blk, core))
    so, co, bo, wo, lo, li = (
        order, core[order], blk[order], w[order], lane[order], lidx[order])
    # position within cell
    cell_sorted = (co * NB + bo) * NW + wo
    cell_start = np.searchsorted(cell_sorted, np.arange(NCORE * NB * NW), side="left")
    rank = np.arange(len(order)) - cell_start[cell_sorted]
    pos = stream_off[bo, wo] + rank
    idx_streams[co, pos] = li.astype(np.int16)
    dst_streams[co, pos] = lo.astype(np.float32)

    # wrapped layouts
    idxw8 = np.zeros((NCORE, 128, TOT // 16), np.int16)
    sbytes = np.zeros((NCORE, 128, TILES, 128), np.uint8)
    tt = np.arange(TOT) // 128
    pp = np.arange(TOT) % 128
    for i in range(NCORE):
        w16 = idx_streams[i].reshape(-1, 16).T  # [16, TOT/16]
        idxw8[i] = np.tile(w16, (8, 1))
        v = dst_streams[i].astype(np.int64)
        m = v >= 0
        sbytes[i, pp[m], tt[m], v[m]] = 0x38  # fp8 e4m3 1.0

    # per-core node-scale vectors [128, NW] (0 on pad rows)
    sv = np.zeros((NCORE, 128, NW), np.float32)
    siv = np.zeros((NCORE, 128, NW), np.float32)
    for i in range(NCORE):
        sp = np.zeros(SHP, np.float32)
        sp[:SH] = s[i * SH : (i + 1) * SH]
        sv[i] = sp.reshape(NW, 128).T
        spi = np.zeros(SHP, np.float32)
        spi[:SH] = 1.0 / s[i * SH : (i + 1) * SH]
        siv[i] = spi.reshape(NW, 128).T
    s2v = sv * sv

    # features transposed + padded per core, bf16
    XT = np.zeros((NCORE, F, SHP), np.float32)
    for i in range(NCORE):
        XT[i, :, :SH] = features[i * SH : (i + 1) * SH].T

    return dict(
        P=P, TOT=TOT, TILES=TILES, gather_info=gather_info,
        stream_off=stream_off, idxw8=idxw8, sbytes=sbytes,
        sv=sv, s2v=s2v, siv=siv, XT=XT,
    )


def _build(prep):
    import concourse.bacc as bacc
    import concourse.bass as bass
    import concourse.tile as tile
    from concourse import mybir

    P = prep["P"]
    TOT = prep["TOT"]
    TILES = prep["TILES"]
    gather_info = prep["gather_info"]
    stream_off = prep["stream_off"]

    dt = mybir.dt
    AF = mybir.ActivationFunctionType
    OP = mybir.AluOpType

    nc = bacc.Bacc(
        "TRN2", target_bir_lowering=False, debug=False,
        num_devices=NCORE, num_swdge_queues=4,
    )

    # ---- parameters ----
    xt_d = nc.dram_tensor("xt", [F, SHP], dt.bfloat16, kind="ExternalInput")
    w1_d = nc.dram_tensor("w1", [F, H], dt.bfloat16, kind="ExternalInput")
    wy_d = nc.dram_tensor("wy", [F, C], dt.bfloat16, kind="ExternalInput")
    w2_d = nc.dram_tensor("w2", [H, C], dt.bfloat16, kind="ExternalInput")
    b1_d = nc.dram_tensor("b1", [H, 1], dt.float32, kind="ExternalInput")
    b2_d = nc.dram_tensor("b2", [128, C], dt.float32, kind="ExternalInput")
    sv_d = nc.dram_tensor("sv", [128, NW], dt.float32, kind="ExternalInput")
    s2_d = nc.dram_tensor("s2", [128, NW], dt.float32, kind="ExternalInput")
    si_d = nc.dram_tensor("si", [128, NW], dt.float32, kind="ExternalInput")
    idx_d = nc.dram_tensor("idx", [128, TOT // 16], dt.int16, kind="ExternalInput")
    sb_d = nc.dram_tensor("sb", [128, TILES, 128], dt.float8e4, kind="ExternalInput")
    out_d = nc.dram_tensor("h_out", [SHP, C], dt.float32, kind="ExternalOutput")

    # ---- internal DRAM ----
    shards = [nc.dram_tensor(f"shard{j}", [SHP, 128], dt.bfloat16) for j in range(2)]
    tabs = [
        nc.dram_tensor(f"tab{j}", [NCORE * SHP, 128], dt.bfloat16, addr_space="Shared")
        for j in range(2)
    ]

    MAXC = int(max(gi[3] for gi in gather_info)) // 128  # max gather cols
    MAXT = int(P.max()) // 128

    with tile.TileContext(nc) as tc:
        with (
            tc.tile_pool(name="cst", bufs=1) as cst,
            tc.tile_pool(name="state", bufs=1) as statep,
            tc.tile_pool(name="dense", bufs=3) as densep,
            tc.tile_pool(name="ebufs", bufs=8) as ebufs,
            tc.tile_pool(name="ibufs", bufs=8) as ibufs,
            tc.tile_pool(name="sbl", bufs=4) as sblp,
            tc.tile_pool(name="scr", bufs=4) as scr,
            tc.tile_pool(name="psum", bufs=1, space="PSUM") as psum,
        ):
            # ---- resident tiles ----
            sv = cst.tile([128, NW], dt.float32)
            s2 = cst.tile([128, NW], dt.float32)
            si = cst.tile([128, NW], dt.float32)
            b1t = cst.tile([H, 1], dt.float32)
            b2t = cst.tile([128, C], dt.float32)
            w1t = cst.tile([128, 4, H], dt.bfloat16)
            wyt = cst.tile([128, 4, C], dt.bfloat16)
            w2t = cst.tile([H, C], dt.bfloat16)
            lgp = statep.tile([128, NW, C], dt.float32)
            hcp = statep.tile([128, NW, C], dt.float32)
            h0p = statep.tile([128, NW, C], dt.float32)
            stage = statep.tile([128, NW, 128], dt.bfloat16)

            nc.sync.dma_start(out=sv[:], in_=sv_d[:])
            nc.sync.dma_start(out=s2[:], in_=s2_d[:])
            nc.sync.dma_start(out=si[:], in_=si_d[:])
            nc.sync.dma_start(out=b1t[:], in_=b1_d[:])
            nc.sync.dma_start(out=b2t[:], in_=b2_d[:])
            nc.sync.dma_start(
                out=w1t[:], in_=w1_d[:].rearrange("(c p) h -> p c h", p=128))
            nc.sync.dma_start(
                out=wyt[:], in_=wy_d[:].rearrange("(c p) h -> p c h", p=128))
            nc.sync.dma_start(out=w2t[:], in_=w2_d[:])

            # ================= dense phase =================
            for nt in range(NW):
                xt_t = densep.tile([128, 4, 128], dt.bfloat16, name="xt_t")
                nc.sync.dma_start(
                    out=xt_t[:],
                    in_=xt_d[:, nt * 128 : (nt + 1) * 128].rearrange(
                        "(c p) n -> p c n", p=128),
                )
                # logits psum [128 nodes, C]
                ps_l = psum.tile([128, C], dt.float32, space="PSUM", name="ps_l", bufs=1)
                for c4 in range(4):
                    nc.tensor.matmul(
                        out=ps_l[:], lhsT=xt_t[:, c4, :], rhs=wyt[:, c4, :],
                        start=(c4 == 0), stop=(c4 == 3))
                # hT psum [128 hid, 128 nodes]
                ps_h = psum.tile([128, 128], dt.float32, space="PSUM", name="ps_h", bufs=2)
                for c4 in range(4):
                    nc.tensor.matmul(
                        out=ps_h[:], lhsT=w1t[:, c4, :], rhs=xt_t[:, c4, :],
                        start=(c4 == 0), stop=(c4 == 3))
                hT_t = scr.tile([128, 128], dt.bfloat16, name="hT_t")
                nc.scalar.activation(out=hT_t[:], in_=ps_h[:], func=AF.Relu,
                                     bias=b1t[:])
                # h0 psum [128 nodes, C]
                ps_h0 = psum.tile([128, C], dt.float32, space="PSUM", name="ps_h0", bufs=1)
                nc.tensor.matmul(out=ps_h0[:], lhsT=hT_t[:], rhs=w2t[:],
                                 start=True, stop=True)
                h0t = scr.tile([128, C], dt.float32, name="h0t")
                nc.vector.tensor_tensor(out=h0t[:], in0=ps_h0[:], in1=b2t[:],
                                        op=OP.add)
                # softmax(logits)
                mx = scr.tile([128, 1], dt.float32, name="mx")
                nc.vector.tensor_reduce(out=mx[:], in_=ps_l[:],
                                        axis=mybir.AxisListType.X, op=OP.max)
                nmx = scr.tile([128, 1], dt.float32, name="nmx")
                nc.vector.tensor_scalar_mul(out=nmx[:], in0=mx[:], scalar1=-1.0)
                et = scr.tile([128, C], dt.float32, name="et")
                sm = scr.tile([128, 1], dt.float32, name="sm")
                nc.scalar.activation(out=et[:], in_=ps_l[:], func=AF.Exp,
                                     bias=nmx[:], accum_out=sm[:])
                rs = scr.tile([128, 1], dt.float32, name="rs")
                nc.vector.reciprocal(out=rs[:], in_=sm[:])
                lgt = scr.tile([128, C], dt.float32, name="lgt")
                nc.vector.tensor_scalar_mul(out=lgt[:], in0=et[:], scalar1=rs[:])
                # scaled state
                nc.vector.tensor_scalar_mul(
                    out=lgp[:, nt, :], in0=lgt[:], scalar1=sv[:, nt : nt + 1])
                nc.vector.tensor_scalar_mul(
                    out=h0p[:, nt, :], in0=h0t[:], scalar1=sv[:, nt : nt + 1])
                nc.vector.tensor_copy(out=hcp[:, nt, :], in_=h0p[:, nt, :])
                nc.vector.tensor_copy(out=stage[:, nt, 0:C], in_=lgp[:, nt, :])
                nc.vector.tensor_copy(out=stage[:, nt, C:128], in_=hcp[:, nt, :])

            shard_ap = lambda j: shards[j][:].rearrange("(w p) f -> p w f", p=128)
            nc.sync.dma_start(out=shard_ap(0), in_=stage[:])
            nc.gpsimd.collective_compute(
                "AllGather", OP.bypass, replica_groups=[list(range(NCORE))],
                ins=[shards[0][:].opt()], outs=[tabs[0][:].opt()])

            # ================= propagation =================
            for k in range(K):
                tab = tabs[k % 2]
                for b in range(NBATCH):
                    ws = list(range(b * WB, min((b + 1) * WB, NW)))
                    ebs = []
                    for B in range(NB):
                        _, _, g_off, g_len = gather_info[b * NB + B]
                        cols = g_len // 128
                        it = ibufs.tile([128, MAXC * 8], dt.int16, name="it")
                        nc.sync.dma_start(
                            out=it[:, : g_len // 16],
                            in_=idx_d[:, g_off // 16 : (g_off + g_len) // 16])
                        eb = ebufs.tile([128, MAXC, 128], dt.bfloat16, name="eb")
                        nc.gpsimd.dma_gather(
                            out_ap=eb[:, :cols, :],
                            in_ap=tab[B * BLK : (B + 1) * BLK, :],
                            idxs_ap=it[:, : g_len // 16],
                            num_idxs=g_len,
                            num_idxs_reg=g_len,
                            elem_size=128,
                            single_packet=False,
                            queue_num=B,
                        )
                        st = sblp.tile([128, MAXC, 128], dt.float8e4, name="st")
                        gt0 = g_off // 128
                        nc.sync.dma_start(
                            out=st[:, :cols, :],
                            in_=sb_d[:, gt0 : gt0 + cols, :],
                        )
                        ebs.append((eb, st))
                    for w_ in ws:
                        ps = psum.tile([128, 128], dt.float32, space="PSUM",
                                       name="ps_w", bufs=4)
                        first = True
                        for B in range(NB):
                            _, _, g_off, g_len = gather_info[b * NB + B]
                            ntl = int(P[B, w_]) // 128
                            toff = (stream_off[B, w_] - g_off) // 128
                            eb_, st_ = ebs[B]
                            for t in range(ntl):
                                nc.tensor.matmul(
                                    out=ps[:],
                                    lhsT=st_[:, toff + t, :],
                                    rhs=eb_[:, toff + t, :],
                                    start=first,
                                    stop=(B == NB - 1 and t == ntl - 1),
                                )
                                first = False
                        # ---- combine for window w_ ----
                        zm = scr.tile([128, C], dt.float32, name="zm")
                        nc.vector.tensor_tensor(
                            out=zm[:], in0=lgp[:, w_, :], in1=ps[:, 0:C],
                            op=OP.mult)
                        zd = scr.tile([128, 1], dt.float32, name="zd")
                        nc.vector.tensor_reduce(
                            out=zd[:], in_=zm[:], axis=mybir.AxisListType.X,
                            op=OP.add)
                        zg = scr.tile([128, 1], dt.float32, name="zg")
                        nc.scalar.activation(out=zg[:], in_=zd[:], func=AF.Sigmoid)
                        # lg' update
                        nc.vector.tensor_scalar_mul(
                            out=lgp[:, w_, :], in0=ps[:, 0:C],
                            scalar1=s2[:, w_ : w_ + 1])
                        # hc' update: z*(s2*Gh - h0p) + h0p
                        t1 = scr.tile([128, C], dt.float32, name="t1")
                        nc.vector.tensor_scalar_mul(
                            out=t1[:], in0=ps[:, C:128],
                            scalar1=s2[:, w_ : w_ + 1])
                        nc.vector.tensor_tensor(
                            out=t1[:], in0=t1[:], in1=h0p[:, w_, :], op=OP.subtract)
                        nc.vector.tensor_scalar_mul(
                            out=t1[:], in0=t1[:], scalar1=zg[:])
                        nc.vector.tensor_tensor(
                            out=hcp[:, w_, :], in0=t1[:], in1=h0p[:, w_, :],
                            op=OP.add)
                        if k < K - 1:
                            nc.vector.tensor_copy(
                                out=stage[:, w_, 0:C], in_=lgp[:, w_, :])
                            nc.vector.tensor_copy(
                                out=stage[:, w_, C:128], in_=hcp[:, w_, :])
                        else:
                            ho = scr.tile([128, C], dt.float32, name="ho")
                            nc.vector.tensor_scalar_mul(
                                out=ho[:], in0=hcp[:, w_, :],
                                scalar1=si[:, w_ : w_ + 1])
                            nc.sync.dma_start(
                                out=out_d[:].rearrange(
                                    "(w p) f -> p w f", p=128)[:, w_, :],
                                in_=ho[:])
                if k < K - 1:
                    sh = shards[(k + 1) % 2]
                    nc.sync.dma_start(
                        out=sh[:].rearrange("(w p) f -> p w f", p=128),
                        in_=stage[:])
                    nc.gpsimd.collective_compute(
                        "AllGather", OP.bypass,
                        replica_groups=[list(range(NCORE))],
                        ins=[sh[:].opt()], outs=[tabs[(k + 1) % 2][:].opt()])

    import time as _time
    _t0 = _time.time()
    print(f"[kernel] graph built, compiling...", flush=True)
    nc.compile()
    print(f"[kernel] nc.compile done {_time.time()-_t0:.1f}s", flush=True)
    return nc


def kernel(features, W1, b1, W2, b2, init_weight_y, edge_index):
    global _last_exec_ns
    from concourse.bass_utils import run_bass_kernel_spmd

    features = np.asarray(features, np.float32)
    W1 = np.asarray(W1, np.float32)
    b1 = np.asarray(b1, np.float32)
    W2 = np.asarray(W2, np.float32)
    b2 = np.asarray(b2, np.float32)
    init_weight_y = np.asarray(init_weight_y, np.float32)
    edge_index = np.asarray(edge_index)

    import time as _time
    _t0 = _time.time()
    prep = _host_prep(features, W1, b1, W2, b2, init_weight_y, edge_index)
    print(f"[kernel] host prep: {_time.time()-_t0:.1f}s TOT={prep['TOT']}", flush=True)
    _t0 = _time.time()
    nc = _build(prep)
    print(f"[kernel] build+compile: {_time.time()-_t0:.1f}s", flush=True)

    b2r = np.tile(b2[None, :], (128, 1)).astype(np.float32)
    in_maps = []
    for i in range(NCORE):
        in_maps.append({
            "xt": _to_bf16_u16(prep["XT"][i]),
            "w1": _to_bf16_u16(W1),
            "wy": _to_bf16_u16(init_weight_y),
            "w2": _to_bf16_u16(W2),
            "b1": np.ascontiguousarray(b1[:, None]).astype(np.float32),
            "b2": b2r,
            "sv": np.ascontiguousarray(prep["sv"][i]),
            "s2": np.ascontiguousarray(prep["s2v"][i]),
            "si": np.ascontiguousarray(prep["siv"][i]),
            "idx": np.ascontiguousarray(prep["idxw8"][i]),
            "sb": np.ascontiguousarray(prep["sbytes"][i]),
        })

    res = run_bass_kernel_spmd(
        nc, in_maps, core_ids=list(range(NCORE)), trace=TRACE)
    _last_exec_ns = res.exec_time_ns

    out = np.empty((N, C), np.float32)
    for i in range(NCORE):
        out[i * SH : (i + 1) * SH] = res.results[i]["h_out"][:SH]
    return out



# revision 6
# speedup vs baseline: 1.0210x; 1.0210x over previous
"""AdaAPPNP (gated APPNP GNN) distributed Bass kernel for 8 TRN2 NeuronCores.

Strategy: node-sharded graph parallelism, 12500 dst nodes per core. Per hop
the gated-APPNP update needs two spmms (A @ lg', A @ hc') over the
symmetric-normalized adjacency; normalization is factored into node scales
s = deg^-1/2 so the edge loop is an unweighted segment-sum:
  spmm_hat(x) = s * (A @ (s * x)).
Both propagated matrices are packed into one bf16 table row [lg' | hc']
(128 feats, 256B) so one gather per edge serves both spmms.

Pipelining layout (the key difference from the naive design): the replicated
table is CHUNK-MAJOR — each core's shard is split into 4 chunks of 25
windows; table block q = [chunk q of core 0 | ... | chunk q of core 7]
(25600 rows, int16-indexable). The per-hop AllGather is split into 4 chunk
collectives, each fired as soon as its 25 windows' combines are done, so
collectives overlap compute and next hop's gathers of block q wait only on
chunk-q's collective. idx/S-matrix loads ride the scalar (ACT) HWDGE queue
so sync-queue ordering never blocks them; shard/output writes use sync (SP).

Per 128-dst window the segment-sum is TensorE: S^T @ E with S a one-hot
edge->dst fp8 matrix (precomputed in DRAM, streamed), E the dma_gathered
source rows. Pad edges get dst -1 so their S column is all zero.
"""

import numpy as np
import ml_dtypes

N = 100000
F = 512
H = 128
C = 64
K = 10
NCORE = 8
SH = 12500            # real nodes per core
NWIN = 100            # 128-dst windows per core (98 real + 2 pad)
SHP = NWIN * 128      # padded shard rows (12800)
CHW = 25              # windows per chunk
CHROWS = CHW * 128    # 3200 rows per chunk per core
NB = 4                # chunks = gather blocks
BLK = NCORE * CHROWS  # 25600 rows per block (int16-safe)
WB = 5                # windows per gather batch
NBATCH = NWIN // WB   # 20
COLL_STAGGER = 3      # batches between chunk-ready and collective issue
import os as _os
COLL_AT_END = _os.environ.get("COLL_AT_END", "0") == "1"

bf16 = ml_dtypes.bfloat16

_last_exec_ns = None
TRACE = False


def _to_bf16_u16(a):
    return np.ascontiguousarray(a.astype(bf16)).view(np.uint16)


def _host_prep(features, W1, b1, W2, b2, init_weight_y, edge_index):
    src = edge_index[0].astype(np.int64)
    dst = edge_index[1].astype(np.int64)
    deg = np.bincount(dst, minlength=N).astype(np.float32)
    s = (1.0 / np.sqrt(np.clip(deg, 1.0, None))).astype(np.float32)

    # chunk-major table row of a source node
    core_s = src // SH
    local_s = src % SH
    q = local_s // CHROWS                       # chunk / gather block
    lidx = core_s * CHROWS + (local_s - q * CHROWS)  # block-local row

    core_d = dst // SH
    local_d = dst - core_d * SH
    w = local_d // 128
    lane = local_d % 128

    # per (dst-core, block, window) cell counts
    cell = (core_d * NB + q) * NWIN + w
    counts = np.bincount(cell, minlength=NCORE * NB * NWIN).reshape(NCORE, NB, NWIN)
    P = (np.ceil(counts.max(axis=0) / 128).astype(np.int64) * 128)  # [NB, NWIN]
    P = np.maximum(P, 128)

    # stream order: batch-major, then block, then window
    stream_off = np.zeros((NB, NWIN), np.int64)
    off = 0
    gather_info = []  # per (batch, B): (b, B, idx_off, idx_len)
    for b in range(NBATCH):
        ws = list(range(b * WB, (b + 1) * WB))
        for B in range(NB):
            g_off = off
            for w_ in ws:
                stream_off[B, w_] = off
                off += int(P[B, w_])
            gather_info.append((b, B, g_off, off - g_off))
    TOT = off
    TILES = TOT // 128

    # fill per-core streams (vectorized by sorting)
    idx_streams = np.zeros((NCORE, TOT), np.int16)   # pad idx 0 (S col kills)
    dst_streams = np.full((NCORE, TOT), -1.0, np.float32)
    order = np.lexsort((lane, w, q, core_d))
    co, bo, wo, lo, li = (
        core_d[order], q[order], w[order], lane[order], lidx[order])
    cell_sorted = (co * NB + bo) * NWIN + wo
    cell_start = np.searchsorted(cell_sorted, np.arange(NCORE * NB * NWIN), side="left")
    rank = np.arange(len(order)) - cell_start[cell_sorted]
    pos = stream_off[bo, wo] + rank
    idx_streams[co, pos] = li.astype(np.int16)
    dst_streams[co, pos] = lo.astype(np.float32)

    # wrapped layouts
    idxw8 = np.zeros((NCORE, 128, TOT // 16), np.int16)
    sbytes = np.zeros((NCORE, 128, TILES, 128), np.uint8)
    tt = np.arange(TOT) // 128
    pp = np.arange(TOT) % 128
    for i in range(NCORE):
        w16 = idx_streams[i].reshape(-1, 16).T  # [16, TOT/16]
        idxw8[i] = np.tile(w16, (8, 1))
        v = dst_streams[i].astype(np.int64)
        m = v >= 0
        sbytes[i, pp[m], tt[m], v[m]] = 0x38  # fp8 e4m3 1.0

    # per-core node-scale vectors [128, NWIN] (0 on pad rows)
    sv = np.zeros((NCORE, 128, NWIN), np.float32)
    siv = np.zeros((NCORE, 128, NWIN), np.float32)
    for i in range(NCORE):
        sp = np.zeros(SHP, np.float32)
        sp[:SH] = s[i * SH : (i + 1) * SH]
        sv[i] = sp.reshape(NWIN, 128).T
        spi = np.zeros(SHP, np.float32)
        spi[:SH] = 1.0 / s[i * SH : (i + 1) * SH]
        siv[i] = spi.reshape(NWIN, 128).T
    s2v = sv * sv

    # features transposed + padded per core, bf16
    XT = np.zeros((NCORE, F, SHP), np.float32)
    for i in range(NCORE):
        XT[i, :, :SH] = features[i * SH : (i + 1) * SH].T

    return dict(
        P=P, TOT=TOT, TILES=TILES, gather_info=gather_info,
        stream_off=stream_off, idxw8=idxw8, sbytes=sbytes,
        sv=sv, s2v=s2v, siv=siv, XT=XT,
    )


def _build(prep):
    import concourse.bacc as bacc
    import concourse.tile as tile
    from concourse import mybir

    P = prep["P"]
    TOT = prep["TOT"]
    gather_info = prep["gather_info"]
    stream_off = prep["stream_off"]

    dt = mybir.dt
    AF = mybir.ActivationFunctionType
    OP = mybir.AluOpType

    nc = bacc.Bacc(
        "TRN2", target_bir_lowering=False, debug=False,
        num_devices=NCORE, num_swdge_queues=4,
    )

    # ---- parameters ----
    xt_d = nc.dram_tensor("xt", [F, SHP], dt.bfloat16, kind="ExternalInput")
    w1_d = nc.dram_tensor("w1", [F, H], dt.bfloat16, kind="ExternalInput")
    wy_d = nc.dram_tensor("wy", [F, C], dt.bfloat16, kind="ExternalInput")
    w2_d = nc.dram_tensor("w2", [H, C], dt.bfloat16, kind="ExternalInput")
    b1_d = nc.dram_tensor("b1", [H, 1], dt.float32, kind="ExternalInput")
    b2_d = nc.dram_tensor("b2", [128, C], dt.float32, kind="ExternalInput")
    sv_d = nc.dram_tensor("sv", [128, NWIN], dt.float32, kind="ExternalInput")
    s2_d = nc.dram_tensor("s2", [128, NWIN], dt.float32, kind="ExternalInput")
    si_d = nc.dram_tensor("si", [128, NWIN], dt.float32, kind="ExternalInput")
    idx_d = nc.dram_tensor("idx", [128, TOT // 16], dt.int16, kind="ExternalInput")
    sb_d = nc.dram_tensor("sb", [128, TOT // 128, 128], dt.float8e4, kind="ExternalInput")
    out_d = nc.dram_tensor("h_out", [SHP, C], dt.float32, kind="ExternalOutput")

    # ---- internal DRAM: per-chunk shards and double-buffered table blocks ----
    shards = [nc.dram_tensor(f"shard{q}", [CHROWS, 128], dt.bfloat16) for q in range(NB)]
    tabs = [
        [
            nc.dram_tensor(f"tab{j}_{q}", [BLK, 128], dt.bfloat16, addr_space="Shared")
            for q in range(NB)
        ]
        for j in range(2)
    ]

    MAXC = int(max(gi[3] for gi in gather_info)) // 128  # max gather cols

    def chunk_flush(tc_nc, stage, c, par_w):
        """DMA stage chunk c -> shard_c, then AllGather into tab[par_w][c]."""
        sh = shards[c]
        tc_nc.sync.dma_start(
            out=sh[:].rearrange("(w p) f -> p w f", p=128),
            in_=stage[:, c * CHW : (c + 1) * CHW, :],
        )
        tc_nc.gpsimd.collective_compute(
            "AllGather", OP.bypass,
            replica_groups=[list(range(NCORE))],
            ins=[sh[:].opt()], outs=[tabs[par_w][c][:].opt()],
        )

    with tile.TileContext(nc) as tc:
        with (
            tc.tile_pool(name="cst", bufs=1) as cst,
            tc.tile_pool(name="state", bufs=1) as statep,
            tc.tile_pool(name="dense", bufs=3) as densep,
            tc.tile_pool(name="ebufs", bufs=6) as ebufs,
            tc.tile_pool(name="ibufs", bufs=6) as ibufs,
            tc.tile_pool(name="sbl", bufs=4) as sblp,
            tc.tile_pool(name="scr", bufs=8) as scr,
            tc.tile_pool(name="psum", bufs=1, space="PSUM") as psum,
        ):
            # ---- resident tiles ----
            sv = cst.tile([128, NWIN], dt.float32)
            s2 = cst.tile([128, NWIN], dt.float32)
            si = cst.tile([128, NWIN], dt.float32)
            b1t = cst.tile([H, 1], dt.float32)
            b2t = cst.tile([128, C], dt.float32)
            w1t = cst.tile([128, 4, H], dt.bfloat16)
            wyt = cst.tile([128, 4, C], dt.bfloat16)
            w2t = cst.tile([H, C], dt.bfloat16)
            lgp = statep.tile([128, NWIN, C], dt.float32)
            h0p = statep.tile([128, NWIN, C], dt.float32)
            stage = statep.tile([128, NWIN, 128], dt.bfloat16)

            nc.sync.dma_start(out=sv[:], in_=sv_d[:])
            nc.sync.dma_start(out=s2[:], in_=s2_d[:])
            nc.sync.dma_start(out=si[:], in_=si_d[:])
            nc.sync.dma_start(out=b1t[:], in_=b1_d[:])
            nc.sync.dma_start(out=b2t[:], in_=b2_d[:])
            nc.sync.dma_start(
                out=w1t[:], in_=w1_d[:].rearrange("(c p) h -> p c h", p=128))
            nc.sync.dma_start(
                out=wyt[:], in_=wy_d[:].rearrange("(c p) h -> p c h", p=128))
            nc.sync.dma_start(out=w2t[:], in_=w2_d[:])

            # ================= dense phase =================
            for nt in range(NWIN):
                xt_t = densep.tile([128, 4, 128], dt.bfloat16, name="xt_t")
                nc.scalar.dma_start(
                    out=xt_t[:],
                    in_=xt_d[:, nt * 128 : (nt + 1) * 128].rearrange(
                        "(c p) n -> p c n", p=128),
                )
                # logits psum [128 nodes, C]
                ps_l = psum.tile([128, C], dt.float32, space="PSUM", name="ps_l", bufs=1)
                for c4 in range(4):
                    nc.tensor.matmul(
                        out=ps_l[:], lhsT=xt_t[:, c4, :], rhs=wyt[:, c4, :],
                        start=(c4 == 0), stop=(c4 == 3))
                # hT psum [128 hid, 128 nodes]
                ps_h = psum.tile([128, 128], dt.float32, space="PSUM", name="ps_h", bufs=2)
                for c4 in range(4):
                    nc.tensor.matmul(
                        out=ps_h[:], lhsT=w1t[:, c4, :], rhs=xt_t[:, c4, :],
                        start=(c4 == 0), stop=(c4 == 3))
                hT_t = scr.tile([128, 128], dt.bfloat16, name="hT_t")
                nc.scalar.activation(out=hT_t[:], in_=ps_h[:], func=AF.Relu,
                                     bias=b1t[:])
                # h0 psum [128 nodes, C]
                ps_h0 = psum.tile([128, C], dt.float32, space="PSUM", name="ps_h0", bufs=1)
                nc.tensor.matmul(out=ps_h0[:], lhsT=hT_t[:], rhs=w2t[:],
                                 start=True, stop=True)
                h0t = scr.tile([128, C], dt.float32, name="h0t")
                nc.vector.tensor_tensor(out=h0t[:], in0=ps_h0[:], in1=b2t[:],
                                        op=OP.add)
                # softmax(logits)
                mx = scr.tile([128, 1], dt.float32, name="mx")
                nc.vector.tensor_reduce(out=mx[:], in_=ps_l[:],
                                        axis=mybir.AxisListType.X, op=OP.max)
                nmx = scr.tile([128, 1], dt.float32, name="nmx")
                nc.vector.tensor_scalar_mul(out=nmx[:], in0=mx[:], scalar1=-1.0)
                et = scr.tile([128, C], dt.float32, name="et")
                sm = scr.tile([128, 1], dt.float32, name="sm")
                nc.scalar.activation(out=et[:], in_=ps_l[:], func=AF.Exp,
                                     bias=nmx[:], accum_out=sm[:])
                rs = scr.tile([128, 1], dt.float32, name="rs")
                nc.vector.reciprocal(out=rs[:], in_=sm[:])
                lgt = scr.tile([128, C], dt.float32, name="lgt")
                nc.vector.tensor_scalar_mul(out=lgt[:], in0=et[:], scalar1=rs[:])
                # scaled state
                nc.vector.tensor_scalar_mul(
                    out=lgp[:, nt, :], in0=lgt[:], scalar1=sv[:, nt : nt + 1])
                nc.vector.tensor_scalar_mul(
                    out=h0p[:, nt, :], in0=h0t[:], scalar1=sv[:, nt : nt + 1])
                nc.vector.tensor_copy(out=stage[:, nt, 0:C], in_=lgp[:, nt, :])
                nc.vector.tensor_copy(out=stage[:, nt, C:128], in_=h0p[:, nt, :])
                if nt % CHW == CHW - 1:
                    chunk_flush(nc, stage, nt // CHW, 0)

            # ================= propagation =================
            for k in range(K):
                par_r = k % 2
                par_w = (k + 1) % 2
                pend = []  # chunks awaiting collective issue: (ready_batch, c)
                for b in range(NBATCH):
                    ws = list(range(b * WB, (b + 1) * WB))
                    ebs = []
                    for B in range(NB):
                        _, _, g_off, g_len = gather_info[b * NB + B]
                        cols = g_len // 128
                        it = ibufs.tile([128, MAXC * 8], dt.int16, name="it")
                        nc.scalar.dma_start(
                            out=it[:, : g_len // 16],
                            in_=idx_d[:, g_off // 16 : (g_off + g_len) // 16])
                        eb = ebufs.tile([128, MAXC, 128], dt.bfloat16, name="eb")
                        nc.gpsimd.dma_gather(
                            out_ap=eb[:, :cols, :],
                            in_ap=tabs[par_r][B][:],
                            idxs_ap=it[:, : g_len // 16],
                            num_idxs=g_len,
                            num_idxs_reg=g_len,
                            elem_size=128,
                            single_packet=False,
                            queue_num=B,
                        )
                        st = sblp.tile([128, MAXC, 128], dt.float8e4, name="st")
                        gt0 = g_off // 128
                        nc.scalar.dma_start(
                            out=st[:, :cols, :],
                            in_=sb_d[:, gt0 : gt0 + cols, :],
                        )
                        ebs.append((eb, st))
                    for w_ in ws:
                        ps = psum.tile([128, 128], dt.float32, space="PSUM",
                                       name="ps_w", bufs=4)
                        first = True
                        for B in range(NB):
                            _, _, g_off, g_len = gather_info[b * NB + B]
                            ntl = int(P[B, w_]) // 128
                            toff = (stream_off[B, w_] - g_off) // 128
                            eb_, st_ = ebs[B]
                            for t in range(ntl):
                                nc.tensor.matmul(
                                    out=ps[:],
                                    lhsT=st_[:, toff + t, :],
                                    rhs=eb_[:, toff + t, :],
                                    start=first,
                                    stop=(B == NB - 1 and t == ntl - 1),
                                )
                                first = False
                        # ---- combine for window w_ ----
                        zm = scr.tile([128, C], dt.float32, name="zm")
                        nc.vector.tensor_tensor(
                            out=zm[:], in0=lgp[:, w_, :], in1=ps[:, 0:C],
                            op=OP.mult)
                        zd = scr.tile([128, 1], dt.float32, name="zd")
                        nc.vector.tensor_reduce(
                            out=zd[:], in_=zm[:], axis=mybir.AxisListType.X,
                            op=OP.add)
                        zg = scr.tile([128, 1], dt.float32, name="zg")
                        nc.scalar.activation(out=zg[:], in_=zd[:], func=AF.Sigmoid)
                        # hc' = z*(s2*Gh - h0p) + h0p
                        t1 = scr.tile([128, C], dt.float32, name="t1")
                        nc.vector.tensor_scalar_mul(
                            out=t1[:], in0=ps[:, C:128],
                            scalar1=s2[:, w_ : w_ + 1])
                        nc.vector.tensor_tensor(
                            out=t1[:], in0=t1[:], in1=h0p[:, w_, :], op=OP.subtract)
                        nc.vector.tensor_scalar_mul(
                            out=t1[:], in0=t1[:], scalar1=zg[:])
                        if k < K - 1:
                            # lg' update: fp32 state + bf16 stage, no cast chain
                            nc.vector.tensor_scalar_mul(
                                out=lgp[:, w_, :], in0=ps[:, 0:C],
                                scalar1=s2[:, w_ : w_ + 1])
                            nc.vector.tensor_scalar_mul(
                                out=stage[:, w_, 0:C], in0=ps[:, 0:C],
                                scalar1=s2[:, w_ : w_ + 1])
                            nc.vector.tensor_tensor(
                                out=stage[:, w_, C:128], in0=t1[:],
                                in1=h0p[:, w_, :], op=OP.add)
                        else:
                            ho = scr.tile([128, C], dt.float32, name="ho")
                            nc.vector.tensor_tensor(
                                out=ho[:], in0=t1[:], in1=h0p[:, w_, :],
                                op=OP.add)
                            ho2 = scr.tile([128, C], dt.float32, name="ho2")
                            nc.vector.tensor_scalar_mul(
                                out=ho2[:], in0=ho[:],
                                scalar1=si[:, w_ : w_ + 1])
                            nc.sync.dma_start(
                                out=out_d[:].rearrange(
                                    "(w p) f -> p w f", p=128)[:, w_, :],
                                in_=ho2[:])
                    # stagger chunk collectives under continuing compute
                    if k < K - 1 and not COLL_AT_END:
                        if (b + 1) % WB == 0:
                            pend.append((b, b // WB))
                        while pend and (pend[0][0] + COLL_STAGGER <= b or b == NBATCH - 1):
                            _, c = pend.pop(0)
                            chunk_flush(nc, stage, c, par_w)
                if k < K - 1 and COLL_AT_END:
                    for c in range(NB):
                        chunk_flush(nc, stage, c, par_w)

    import time as _time
    _t0 = _time.time()
    print(f"[kernel] graph built, compiling...", flush=True)
    nc.compile()
    print(f"[kernel] nc.compile done {_time.time()-_t0:.1f}s", flush=True)
    return nc


def kernel(features, W1, b1, W2, b2, init_weight_y, edge_index):
    global _last_exec_ns
    from concourse.bass_utils import run_bass_kernel_spmd

    features = np.asarray(features, np.float32)
    W1 = np.asarray(W1, np.float32)
    b1 = np.asarray(b1, np.float32)
    W2 = np.asarray(W2, np.float32)
    b2 = np.asarray(b2, np.float32)
    init_weight_y = np.asarray(init_weight_y, np.float32)
    edge_index = np.asarray(edge_index)

    import time as _time
    _t0 = _time.time()
    prep = _host_prep(features, W1, b1, W2, b2, init_weight_y, edge_index)
    print(f"[kernel] host prep: {_time.time()-_t0:.1f}s TOT={prep['TOT']}", flush=True)
    _t0 = _time.time()
    nc = _build(prep)
    print(f"[kernel] build+compile: {_time.time()-_t0:.1f}s", flush=True)

    b2r = np.tile(b2[None, :], (128, 1)).astype(np.float32)
    in_maps = []
    for i in range(NCORE):
        in_maps.append({
            "xt": _to_bf16_u16(prep["XT"][i]),
            "w1": _to_bf16_u16(W1),
            "wy": _to_bf16_u16(init_weight_y),
            "w2": _to_bf16_u16(W2),
            "b1": np.ascontiguousarray(b1[:, None]).astype(np.float32),
            "b2": b2r,
            "sv": np.ascontiguousarray(prep["sv"][i]),
            "s2": np.ascontiguousarray(prep["s2v"][i]),
            "si": np.ascontiguousarray(prep["siv"][i]),
            "idx": np.ascontiguousarray(prep["idxw8"][i]),
            "sb": np.ascontiguousarray(prep["sbytes"][i]),
        })

    res = run_bass_kernel_spmd(
        nc, in_maps, core_ids=list(range(NCORE)), trace=TRACE)
    _last_exec_ns = res.exec_time_ns

    out = np.empty((N, C), np.float32)
    for i in range(NCORE):
        out[i * SH : (i + 1) * SH] = res.results[i]["h_out"][:SH]
    return out


# revision 9
# speedup vs baseline: 1.3197x; 1.2926x over previous
"""AdaAPPNP (gated APPNP GNN) distributed Bass kernel for 8 TRN2 NeuronCores.

Strategy: node-sharded graph parallelism, 12500 dst nodes per core. Per hop
the gated-APPNP update needs two spmms (A @ lg', A @ hc') over the
symmetric-normalized adjacency; normalization is factored into node scales
s = deg^-1/2 so the edge loop is an unweighted segment-sum:
  spmm_hat(x) = s * (A @ (s * x)).
Both propagated matrices are packed into one bf16 table row [lg' | hc']
(128 feats, 256B) so one gather per edge serves both spmms.

Pipelining layout (the key difference from the naive design): the replicated
table is CHUNK-MAJOR — each core's shard is split into 4 chunks of 25
windows; table block q = [chunk q of core 0 | ... | chunk q of core 7]
(25600 rows, int16-indexable). The per-hop AllGather is split into 4 chunk
collectives, each fired as soon as its 25 windows' combines are done, so
collectives overlap compute and next hop's gathers of block q wait only on
chunk-q's collective. idx/S-matrix loads ride the scalar (ACT) HWDGE queue
so sync-queue ordering never blocks them; shard/output writes use sync (SP).

Per 128-dst window the segment-sum is TensorE: S^T @ E with S a one-hot
edge->dst fp8 matrix (precomputed in DRAM, streamed), E the dma_gathered
source rows. Pad edges get dst -1 so their S column is all zero.
"""

import numpy as np
import ml_dtypes

N = 100000
F = 512
H = 128
C = 64
K = 10
NCORE = 8
SH = 12500            # real nodes per core
NWIN = 100            # 128-dst windows per core (98 real + 2 pad)
SHP = NWIN * 128      # padded shard rows (12800)
CHW = 25              # windows per chunk
CHROWS = CHW * 128    # 3200 rows per chunk per core
NB = 4                # chunks = gather blocks
BLK = NCORE * CHROWS  # 25600 rows per block (int16-safe)
WB = 4                # windows per gather batch
NBATCH = NWIN // WB   # 20
COLL_STAGGER = 3      # batches between chunk-ready and collective issue
import os as _os
COLL_AT_END = _os.environ.get("COLL_AT_END", "0") == "1"

bf16 = ml_dtypes.bfloat16

_last_exec_ns = None
TRACE = False


def _to_bf16_u16(a):
    return np.ascontiguousarray(a.astype(bf16)).view(np.uint16)


def _host_prep(features, W1, b1, W2, b2, init_weight_y, edge_index):
    src = edge_index[0].astype(np.int64)
    dst = edge_index[1].astype(np.int64)
    deg = np.bincount(dst, minlength=N).astype(np.float32)
    s = (1.0 / np.sqrt(np.clip(deg, 1.0, None))).astype(np.float32)

    # chunk-major table row of a source node
    core_s = src // SH
    local_s = src % SH
    q = local_s // CHROWS                       # chunk / gather block
    lidx = core_s * CHROWS + (local_s - q * CHROWS)  # block-local row

    core_d = dst // SH
    local_d = dst - core_d * SH
    w = local_d // 128
    lane = local_d % 128

    # per (dst-core, block, window) cell counts
    cell = (core_d * NB + q) * NWIN + w
    counts = np.bincount(cell, minlength=NCORE * NB * NWIN).reshape(NCORE, NB, NWIN)
    P = (np.ceil(counts.max(axis=0) / 128).astype(np.int64) * 128)  # [NB, NWIN]
    P = np.maximum(P, 128)

    # stream order: batch-major, then block, then window
    stream_off = np.zeros((NB, NWIN), np.int64)
    off = 0
    gather_info = []  # per (batch, B): (b, B, idx_off, idx_len)
    for b in range(NBATCH):
        ws = list(range(b * WB, (b + 1) * WB))
        for B in range(NB):
            g_off = off
            for w_ in ws:
                stream_off[B, w_] = off
                off += int(P[B, w_])
            gather_info.append((b, B, g_off, off - g_off))
    TOT = off
    TILES = TOT // 128

    # fill per-core streams (vectorized by sorting)
    idx_streams = np.zeros((NCORE, TOT), np.int16)   # pad idx 0 (S col kills)
    dst_streams = np.full((NCORE, TOT), -1.0, np.float32)
    order = np.lexsort((lane, w, q, core_d))
    co, bo, wo, lo, li = (
        core_d[order], q[order], w[order], lane[order], lidx[order])
    cell_sorted = (co * NB + bo) * NWIN + wo
    cell_start = np.searchsorted(cell_sorted, np.arange(NCORE * NB * NWIN), side="left")
    rank = np.arange(len(order)) - cell_start[cell_sorted]
    pos = stream_off[bo, wo] + rank
    idx_streams[co, pos] = li.astype(np.int16)
    dst_streams[co, pos] = lo.astype(np.float32)

    # wrapped layouts
    idxw8 = np.zeros((NCORE, 128, TOT // 16), np.int16)
    sbytes = np.zeros((NCORE, 128, TILES, 128), np.uint8)
    tt = np.arange(TOT) // 128
    pp = np.arange(TOT) % 128
    for i in range(NCORE):
        w16 = idx_streams[i].reshape(-1, 16).T  # [16, TOT/16]
        idxw8[i] = np.tile(w16, (8, 1))
        v = dst_streams[i].astype(np.int64)
        m = v >= 0
        sbytes[i, pp[m], tt[m], v[m]] = 0x38  # fp8 e4m3 1.0

    # per-core node-scale vectors [128, NWIN] (0 on pad rows)
    sv = np.zeros((NCORE, 128, NWIN), np.float32)
    siv = np.zeros((NCORE, 128, NWIN), np.float32)
    for i in range(NCORE):
        sp = np.zeros(SHP, np.float32)
        sp[:SH] = s[i * SH : (i + 1) * SH]
        sv[i] = sp.reshape(NWIN, 128).T
        spi = np.zeros(SHP, np.float32)
        spi[:SH] = 1.0 / s[i * SH : (i + 1) * SH]
        siv[i] = spi.reshape(NWIN, 128).T
    s2v = sv * sv

    # features transposed + padded per core, bf16
    XT = np.zeros((NCORE, F, SHP), np.float32)
    for i in range(NCORE):
        XT[i, :, :SH] = features[i * SH : (i + 1) * SH].T

    return dict(
        P=P, TOT=TOT, TILES=TILES, gather_info=gather_info,
        stream_off=stream_off, idxw8=idxw8, sbytes=sbytes,
        sv=sv, s2v=s2v, siv=siv, XT=XT,
    )


def _build(prep):
    import concourse.bacc as bacc
    import concourse.tile as tile
    from concourse import mybir

    P = prep["P"]
    TOT = prep["TOT"]
    gather_info = prep["gather_info"]
    stream_off = prep["stream_off"]

    dt = mybir.dt
    AF = mybir.ActivationFunctionType
    OP = mybir.AluOpType

    nc = bacc.Bacc(
        "TRN2", target_bir_lowering=False, debug=False,
        num_devices=NCORE, num_swdge_queues=4,
    )

    # ---- parameters ----
    xt_d = nc.dram_tensor("xt", [F, SHP], dt.bfloat16, kind="ExternalInput")
    w1_d = nc.dram_tensor("w1", [F, H], dt.bfloat16, kind="ExternalInput")
    wy_d = nc.dram_tensor("wy", [F, C], dt.bfloat16, kind="ExternalInput")
    w2_d = nc.dram_tensor("w2", [H, C], dt.bfloat16, kind="ExternalInput")
    b1_d = nc.dram_tensor("b1", [H, 1], dt.float32, kind="ExternalInput")
    b2_d = nc.dram_tensor("b2", [128, C], dt.float32, kind="ExternalInput")
    sv_d = nc.dram_tensor("sv", [128, NWIN], dt.float32, kind="ExternalInput")
    s2_d = nc.dram_tensor("s2", [128, NWIN], dt.float32, kind="ExternalInput")
    si_d = nc.dram_tensor("si", [128, NWIN], dt.float32, kind="ExternalInput")
    idx_d = nc.dram_tensor("idx", [128, TOT // 16], dt.int16, kind="ExternalInput")
    sb_d = nc.dram_tensor("sb", [128, TOT // 128, 128], dt.float8e4, kind="ExternalInput")
    out_d = nc.dram_tensor("h_out", [SHP, C], dt.float32, kind="ExternalOutput")

    # ---- internal DRAM: per-chunk shards and double-buffered table blocks ----
    shards = [nc.dram_tensor(f"shard{q}", [CHROWS, 128], dt.bfloat16) for q in range(NB)]
    tabs = [
        [
            nc.dram_tensor(f"tab{j}_{q}", [BLK, 128], dt.bfloat16, addr_space="Shared")
            for q in range(NB)
        ]
        for j in range(2)
    ]

    MAXC = int(max(gi[3] for gi in gather_info)) // 128  # max gather cols

    def chunk_flush(tc_nc, stage, c, par_w):
        """DMA stage chunk c -> shard_c, then AllGather into tab[par_w][c]."""
        sh = shards[c]
        tc_nc.sync.dma_start(
            out=sh[:].rearrange("(w p) f -> p w f", p=128),
            in_=stage[:, c * CHW : (c + 1) * CHW, :],
        )
        tc_nc.gpsimd.collective_compute(
            "AllGather", OP.bypass,
            replica_groups=[list(range(NCORE))],
            ins=[sh[:].opt()], outs=[tabs[par_w][c][:].opt()],
        )

    with tile.TileContext(nc) as tc:
        with (
            tc.tile_pool(name="cst", bufs=1) as cst,
            tc.tile_pool(name="state", bufs=1) as statep,
            tc.tile_pool(name="dense", bufs=3) as densep,
            tc.tile_pool(name="ebufs", bufs=8) as ebufs,
            tc.tile_pool(name="ibufs", bufs=8) as ibufs,
            tc.tile_pool(name="sbl", bufs=6) as sblp,
            tc.tile_pool(name="scr", bufs=6) as scr,
            tc.tile_pool(name="psum", bufs=1, space="PSUM") as psum,
        ):
            # ---- resident tiles ----
            sv = cst.tile([128, NWIN], dt.float32)
            s2 = cst.tile([128, NWIN], dt.float32)
            si = cst.tile([128, NWIN], dt.float32)
            b1t = cst.tile([H, 1], dt.float32)
            b2t = cst.tile([128, C], dt.float32)
            w1t = cst.tile([128, 4, H], dt.bfloat16)
            wyt = cst.tile([128, 4, C], dt.bfloat16)
            w2t = cst.tile([H, C], dt.bfloat16)
            h0p = statep.tile([128, NWIN, C], dt.float32)
            stage = statep.tile([128, NWIN, 128], dt.bfloat16)

            nc.sync.dma_start(out=sv[:], in_=sv_d[:])
            nc.sync.dma_start(out=s2[:], in_=s2_d[:])
            nc.sync.dma_start(out=si[:], in_=si_d[:])
            nc.sync.dma_start(out=b1t[:], in_=b1_d[:])
            nc.sync.dma_start(out=b2t[:], in_=b2_d[:])
            nc.sync.dma_start(
                out=w1t[:], in_=w1_d[:].rearrange("(c p) h -> p c h", p=128))
            nc.sync.dma_start(
                out=wyt[:], in_=wy_d[:].rearrange("(c p) h -> p c h", p=128))
            nc.sync.dma_start(out=w2t[:], in_=w2_d[:])

            # ================= dense phase =================
            for nt in range(NWIN):
                xt_t = densep.tile([128, 4, 128], dt.bfloat16, name="xt_t")
                nc.scalar.dma_start(
                    out=xt_t[:],
                    in_=xt_d[:, nt * 128 : (nt + 1) * 128].rearrange(
                        "(c p) n -> p c n", p=128),
                )
                # logits psum [128 nodes, C]
                ps_lt = psum.tile([128, 128], dt.float32, space="PSUM", name="ps_w", bufs=8)
                ps_l = ps_lt[:, 0:C]
                for c4 in range(4):
                    nc.tensor.matmul(
                        out=ps_l[:], lhsT=xt_t[:, c4, :], rhs=wyt[:, c4, :],
                        start=(c4 == 0), stop=(c4 == 3))
                # hT psum [128 hid, 128 nodes]
                ps_h = psum.tile([128, 128], dt.float32, space="PSUM", name="ps_w", bufs=8)
                for c4 in range(4):
                    nc.tensor.matmul(
                        out=ps_h[:], lhsT=w1t[:, c4, :], rhs=xt_t[:, c4, :],
                        start=(c4 == 0), stop=(c4 == 3))
                hT_t = scr.tile([128, 128], dt.bfloat16, name="hT_t")
                nc.scalar.activation(out=hT_t[:], in_=ps_h[:], func=AF.Relu,
                                     bias=b1t[:])
                # h0 psum [128 nodes, C]
                ps_h0t = psum.tile([128, 128], dt.float32, space="PSUM", name="ps_w", bufs=8)
                ps_h0 = ps_h0t[:, 0:C]
                nc.tensor.matmul(out=ps_h0[:], lhsT=hT_t[:], rhs=w2t[:],
                                 start=True, stop=True)
                h0t = scr.tile([128, C], dt.float32, name="h0t")
                nc.vector.tensor_tensor(out=h0t[:], in0=ps_h0[:], in1=b2t[:],
                                        op=OP.add)
                # softmax(logits)
                mx = scr.tile([128, 1], dt.float32, name="mx")
                nc.vector.tensor_reduce(out=mx[:], in_=ps_l[:],
                                        axis=mybir.AxisListType.X, op=OP.max)
                nmx = scr.tile([128, 1], dt.float32, name="nmx")
                nc.vector.tensor_scalar_mul(out=nmx[:], in0=mx[:], scalar1=-1.0)
                et = scr.tile([128, C], dt.float32, name="et")
                sm = scr.tile([128, 1], dt.float32, name="sm")
                nc.scalar.activation(out=et[:], in_=ps_l[:], func=AF.Exp,
                                     bias=nmx[:], accum_out=sm[:])
                rs = scr.tile([128, 1], dt.float32, name="rs")
                nc.vector.reciprocal(out=rs[:], in_=sm[:])
                lgt = scr.tile([128, C], dt.float32, name="lgt")
                nc.vector.tensor_scalar_mul(out=lgt[:], in0=et[:], scalar1=rs[:])
                # scaled state
                nc.vector.tensor_scalar_mul(
                    out=stage[:, nt, 0:C], in0=lgt[:], scalar1=sv[:, nt : nt + 1])
                nc.vector.tensor_scalar_mul(
                    out=h0p[:, nt, :], in0=h0t[:], scalar1=sv[:, nt : nt + 1])
                nc.vector.tensor_copy(out=stage[:, nt, C:128], in_=h0p[:, nt, :])
                if nt % CHW == CHW - 1:
                    chunk_flush(nc, stage, nt // CHW, 0)

            # ================= propagation =================
            for k in range(K):
                par_r = k % 2
                par_w = (k + 1) % 2
                pend = []  # chunks awaiting collective issue: (ready_batch, c)
                next_c = 0  # next chunk whose windows are not yet all combined
                for b in range(NBATCH):
                    ws = list(range(b * WB, (b + 1) * WB))
                    ebs = []
                    for B in range(NB):
                        _, _, g_off, g_len = gather_info[b * NB + B]
                        cols = g_len // 128
                        it = ibufs.tile([128, MAXC * 8], dt.int16, name="it")
                        nc.scalar.dma_start(
                            out=it[:, : g_len // 16],
                            in_=idx_d[:, g_off // 16 : (g_off + g_len) // 16])
                        eb = ebufs.tile([128, MAXC, 128], dt.bfloat16, name="eb")
                        nc.gpsimd.dma_gather(
                            out_ap=eb[:, :cols, :],
                            in_ap=tabs[par_r][B][:],
                            idxs_ap=it[:, : g_len // 16],
                            num_idxs=g_len,
                            num_idxs_reg=g_len,
                            elem_size=128,
                            single_packet=False,
                            queue_num=B,
                        )
                        st = sblp.tile([128, MAXC, 128], dt.float8e4, name="st")
                        gt0 = g_off // 128
                        nc.scalar.dma_start(
                            out=st[:, :cols, :],
                            in_=sb_d[:, gt0 : gt0 + cols, :],
                        )
                        ebs.append((eb, st))
                    for w_ in ws:
                        ps = psum.tile([128, 128], dt.float32, space="PSUM",
                                       name="ps_w", bufs=8)
                        first = True
                        for B in range(NB):
                            _, _, g_off, g_len = gather_info[b * NB + B]
                            ntl = int(P[B, w_]) // 128
                            toff = (stream_off[B, w_] - g_off) // 128
                            eb_, st_ = ebs[B]
                            for t in range(ntl):
                                nc.tensor.matmul(
                                    out=ps[:],
                                    lhsT=st_[:, toff + t, :],
                                    rhs=eb_[:, toff + t, :],
                                    start=first,
                                    stop=(B == NB - 1 and t == ntl - 1),
                                )
                                first = False
                        # ---- combine for window w_ ----
                        zm = scr.tile([128, C], dt.float32, name="zm")
                        nc.vector.tensor_tensor(
                            out=zm[:], in0=stage[:, w_, 0:C], in1=ps[:, 0:C],
                            op=OP.mult)
                        zd = scr.tile([128, 1], dt.float32, name="zd")
                        nc.vector.tensor_reduce(
                            out=zd[:], in_=zm[:], axis=mybir.AxisListType.X,
                            op=OP.add)
                        zg = scr.tile([128, 1], dt.float32, name="zg")
                        nc.scalar.activation(out=zg[:], in_=zd[:], func=AF.Sigmoid)
                        # hc' = z*(s2*Gh - h0p) + h0p
                        t1 = scr.tile([128, C], dt.float32, name="t1")
                        nc.vector.tensor_scalar_mul(
                            out=t1[:], in0=ps[:, C:128],
                            scalar1=s2[:, w_ : w_ + 1])
                        nc.vector.tensor_tensor(
                            out=t1[:], in0=t1[:], in1=h0p[:, w_, :], op=OP.subtract)
                        nc.vector.tensor_scalar_mul(
                            out=t1[:], in0=t1[:], scalar1=zg[:])
                        if k < K - 1:
                            nc.vector.tensor_scalar_mul(
                                out=stage[:, w_, 0:C], in0=ps[:, 0:C],
                                scalar1=s2[:, w_ : w_ + 1])
                            nc.vector.tensor_tensor(
                                out=stage[:, w_, C:128], in0=t1[:],
                                in1=h0p[:, w_, :], op=OP.add)
                        else:
                            ho = scr.tile([128, C], dt.float32, name="ho")
                            nc.vector.tensor_tensor(
                                out=ho[:], in0=t1[:], in1=h0p[:, w_, :],
                                op=OP.add)
                            ho2 = scr.tile([128, C], dt.float32, name="ho2")
                            nc.vector.tensor_scalar_mul(
                                out=ho2[:], in0=ho[:],
                                scalar1=si[:, w_ : w_ + 1])
                            nc.sync.dma_start(
                                out=out_d[:].rearrange(
                                    "(w p) f -> p w f", p=128)[:, w_, :],
                                in_=ho2[:])
                    # stagger chunk collectives under continuing compute
                    if k < K - 1 and not COLL_AT_END:
                        while next_c < NB and (b + 1) * WB >= (next_c + 1) * CHW:
                            pend.append((b, next_c))
                            next_c += 1
                        while pend and (pend[0][0] + COLL_STAGGER <= b or b == NBATCH - 1):
                            _, c = pend.pop(0)
                            chunk_flush(nc, stage, c, par_w)
                if k < K - 1 and COLL_AT_END:
                    for c in range(NB):
                        chunk_flush(nc, stage, c, par_w)

    import time as _time
    _t0 = _time.time()
    print(f"[kernel] graph built, compiling...", flush=True)
    nc.compile()
    print(f"[kernel] nc.compile done {_time.time()-_t0:.1f}s", flush=True)
    return nc


def kernel(features, W1, b1, W2, b2, init_weight_y, edge_index):
    global _last_exec_ns
    from concourse.bass_utils import run_bass_kernel_spmd

    features = np.asarray(features, np.float32)
    W1 = np.asarray(W1, np.float32)
    b1 = np.asarray(b1, np.float32)
    W2 = np.asarray(W2, np.float32)
    b2 = np.asarray(b2, np.float32)
    init_weight_y = np.asarray(init_weight_y, np.float32)
    edge_index = np.asarray(edge_index)

    import time as _time
    _t0 = _time.time()
    prep = _host_prep(features, W1, b1, W2, b2, init_weight_y, edge_index)
    print(f"[kernel] host prep: {_time.time()-_t0:.1f}s TOT={prep['TOT']}", flush=True)
    _t0 = _time.time()
    nc = _build(prep)
    print(f"[kernel] build+compile: {_time.time()-_t0:.1f}s", flush=True)

    b2r = np.tile(b2[None, :], (128, 1)).astype(np.float32)
    in_maps = []
    for i in range(NCORE):
        in_maps.append({
            "xt": _to_bf16_u16(prep["XT"][i]),
            "w1": _to_bf16_u16(W1),
            "wy": _to_bf16_u16(init_weight_y),
            "w2": _to_bf16_u16(W2),
            "b1": np.ascontiguousarray(b1[:, None]).astype(np.float32),
            "b2": b2r,
            "sv": np.ascontiguousarray(prep["sv"][i]),
            "s2": np.ascontiguousarray(prep["s2v"][i]),
            "si": np.ascontiguousarray(prep["siv"][i]),
            "idx": np.ascontiguousarray(prep["idxw8"][i]),
            "sb": np.ascontiguousarray(prep["sbytes"][i]),
        })

    res = run_bass_kernel_spmd(
        nc, in_maps, core_ids=list(range(NCORE)), trace=TRACE)
    _last_exec_ns = res.exec_time_ns

    out = np.empty((N, C), np.float32)
    for i in range(NCORE):
        out[i * SH : (i + 1) * SH] = res.results[i]["h_out"][:SH]
    return out
